# revision 1
# baseline (speedup 1.0000x reference)
"""Trainium2 Bass kernel for nn_GroupQueryAttention_51616916963669.

GQA with YaRN RoPE, sliding-window (128) + causal mask, learned sink logit,
qkv/out projections. B=1, S=2048, E=2048, H=32, G=8, D=64.

Sharding over 8 NeuronCores: 2-way sequence (1024 queries each, with a
128-token KV halo) x 4-way heads (8 q-heads / 2 kv-groups each). Each core
computes a partial out-projection (over its 512 ctx dims); the host sums the
4 head-partials per sequence half and concatenates.

v4 design notes:
- Inputs arrive in ~12 large DMAs split across the two HWDGE queues (SP +
  ACT) -- per-chunk DMAs cost ~600ns dispatch each on the sync queue and
  serialized startup by ~20us in v3.
- Everything on the Q/K/V/probs path is bf16 (f32 only inside PSUM): DVE
  runs 2 elem/cycle/lane at 16 bit, and bf16 matmuls avoid the f32r
  partition-offset/PSUM-bank hazard.
- qkv matmul streams 3x384 columns per stationary load; RoPE rotate-half is
  a PE permutation matmul; the cos/sin multiplies run on DVE and the final
  adds (and KR partition-swap) on the otherwise idle GpSimd engine.
- Softmax: exp on ACT straight out of PSUM (2 ops per 4-head group); masked
  row-sums via fused DVE scalar_tensor_tensor with accum_out; normalization
  is one broadcast tensor_tensor; probs transposes on PE with the PSUM->SBUF
  copy on ACT; ctx parity copies on GpSimd.
- Out-projection is emitted after attention (engine queues overlap it with
  the attention tail), streams both query halves per stationary load, and
  writes bf16 via 16 output DMAs split across both queues.
"""
import os
import numpy as np

# ---- problem constants (hardcoded per contract) ----
B, S, E = 1, 2048, 2048
H, G, D = 32, 8, 64
SW = 128
ROPE_BASE = 10000.0
ORIG_CTX = 4096.0
YARN_SCALE = 2.0
BETA_FAST, BETA_SLOW = 32.0, 1.0

# ---- sharding constants ----
NCORES = 8
TOK = 1152           # local kv tokens (9 blocks of 128)
NQ = 1024            # local query tokens (kv blocks 1..8)
QH = 8               # q heads per core
KG = 2               # kv groups per core
FTOT = QH * D + 2 * KG * D   # 768, feature order [K, V, Q0..Q3]
NE = E // 128        # 16 e-chunks
TCH = 384            # qkv matmul N-chunk
NT = TOK // TCH      # 3
SCALE = 1.0 / (D ** 0.5)

_compiled = None


def _build_bass():
    import concourse.bacc as bacc
    import concourse.tile as tile
    import concourse.mybir as mybir
    from concourse.masks import make_identity

    f32 = mybir.dt.float32
    bf16 = mybir.dt.bfloat16
    Exp = mybir.ActivationFunctionType.Exp
    Ident = mybir.ActivationFunctionType.Identity
    Alu = mybir.AluOpType

    nc = bacc.Bacc("TRN2", target_bir_lowering=False, debug=False,
                   num_devices=NCORES)

    # all big tensors are host-packed partition-major so every DMA moves
    # multi-KB contiguous runs per partition (the DGE is packet-rate bound)
    xT = nc.dram_tensor("xT", [128, NE, TOK], bf16, kind="ExternalInput").ap()
    wqkvT = nc.dram_tensor("wqkvT", [128, 3, NE, 256], bf16,
                           kind="ExternalInput").ap()
    bqkvT = nc.dram_tensor("bqkvT", [128, FTOT // 128], f32, kind="ExternalInput").ap()
    woutT = nc.dram_tensor("woutT", [128, 4, E], bf16, kind="ExternalInput").ap()
    tabsD = nc.dram_tensor("tabs", [128, 4, TOK], bf16, kind="ExternalInput").ap()
    masksD = nc.dram_tensor("masks", [128, 2, 256], bf16, kind="ExternalInput").ap()
    esinkD = nc.dram_tensor("esink", [128, QH], f32, kind="ExternalInput").ap()
    permD = nc.dram_tensor("perm", [128, 128], bf16, kind="ExternalInput").ap()
    outT = nc.dram_tensor("outT", [128, NE, NQ], bf16, kind="ExternalOutput").ap()

    with tile.TileContext(nc) as tc:
        from contextlib import ExitStack
        es = ExitStack()
        with es:
            persist = es.enter_context(tc.tile_pool(name="persist", bufs=1))
            qk_pool = es.enter_context(tc.tile_pool(name="qk", bufs=1))
            ctx_pool = es.enter_context(tc.tile_pool(name="ctx", bufs=1))
            qkv_pool = es.enter_context(tc.tile_pool(name="qkv", bufs=1))
            inp_pool = es.enter_context(tc.tile_pool(name="inp", bufs=1))

            # ---- inputs: few big DMAs, split across both HWDGE queues ----
            W_sb = inp_pool.tile([128, 3, NE, 256], bf16)
            x_sb = inp_pool.tile([128, NE, TOK], bf16)
            for fg in range(3):
                nc.sync.dma_start(W_sb[:, fg], wqkvT[:, fg])
            tabs = persist.tile([128, 4, TOK], bf16)
            for eq in range(4):
                nc.scalar.dma_start(x_sb[:, 4 * eq:4 * eq + 4, :],
                                    xT[:, 4 * eq:4 * eq + 4, :])
                if eq == 1:
                    nc.scalar.dma_start(tabs, tabsD)
            cs_t = {"cK": tabs[:, 0, :], "sK": tabs[:, 1, :],
                    "cQ": tabs[:, 2, :], "sQ": tabs[:, 3, :]}

            ident = persist.tile([128, 128], f32)
            make_identity(nc, ident)
            identb = persist.tile([128, 128], bf16)
            nc.vector.tensor_copy(identb, ident)
            b_sb = persist.tile([128, FTOT // 128], f32)
            nc.sync.dma_start(b_sb, bqkvT)
            masks2 = persist.tile([128, 2, 256], bf16)
            nc.sync.dma_start(masks2, masksD)
            es_sink = persist.tile([128, QH], f32)
            nc.sync.dma_start(es_sink, esinkD)
            perm = persist.tile([128, 128], bf16)
            nc.sync.dma_start(perm, permD)
            Wo = persist.tile([128, 4, E], bf16)
            nc.sync.dma_start(Wo, woutT)

            # qkv projection results (feature blocks: 0=K, 1=V, 2..5=Q0..Q3)
            qkvT_t = [qkv_pool.tile([128, TOK], bf16, tag=f"qkvT{i}",
                                    name=f"qkvT{i}") for i in range(6)]
            QR = [qk_pool.tile([128, TOK], bf16, tag=f"QR{i}", name=f"QR{i}")
                  for i in range(4)]
            KR = qk_pool.tile([128, TOK], bf16, tag="KR")
            KRsw = qk_pool.tile([128, TOK], bf16, tag="KRsw")
            Vtok = ctx_pool.tile([128, 9, KG, D], bf16)
            # ctx transposed: [128 part = pair of heads, pair-idx 4, q-half 2, 512]
            ctxT = ctx_pool.tile([128, 4, 2, 512], bf16)

            esA = ExitStack()
            psA = esA.enter_context(
                tc.tile_pool(name="psA", bufs=2, space="PSUM"))
            psR = esA.enter_context(
                tc.tile_pool(name="psR", bufs=2, space="PSUM"))
            rope_sc = es.enter_context(tc.tile_pool(name="ropesc", bufs=3))

            def qkv_block(f):
                """Accumulate feature block f over all 16 e-chunks; one
                stationary load per (e), streaming 3x384 columns."""
                pst = [psA.tile([128, TCH], f32, tag=f"mmA{t}", name=f"mmA{t}")
                       for t in range(NT)]
                fg, fh = f // 2, f % 2
                for e in range(NE):
                    for t in range(NT):
                        nc.tensor.matmul(
                            pst[t], W_sb[:, fg, e, 128 * fh:128 * (fh + 1)],
                            x_sb[:, e, TCH * t:TCH * (t + 1)],
                            start=(e == 0), stop=(e == NE - 1))
                for t in range(NT):
                    nc.scalar.activation(
                        out=qkvT_t[f][:, TCH * t:TCH * (t + 1)], in_=pst[t],
                        func=Ident, bias=b_sb[:, f:f + 1])

            def rope(f, cT, sT, dst, also_swap=None):
                """dst = qkvT[f]*cos + (perm @ qkvT[f])*sinS, in 384-col
                chunks. Rotate-half on PE; muls on DVE; adds on GpSimd."""
                src = qkvT_t[f]
                for t in range(NT):
                    cs_ = slice(TCH * t, TCH * (t + 1))
                    rot = psR.tile([128, TCH], f32, tag="rot", name="rot")
                    nc.tensor.matmul(rot, perm, src[:, cs_],
                                     start=True, stop=True)
                    m1 = rope_sc.tile([128, TCH], bf16, tag="m1", name="m1")
                    nc.vector.tensor_mul(m1, src[:, cs_], cT[:, cs_])
                    m2 = rope_sc.tile([128, TCH], bf16, tag="m2", name="m2")
                    nc.vector.tensor_mul(m2, rot, sT[:, cs_])
                    nc.gpsimd.tensor_add(dst[:, cs_], m1, m2)
                    if also_swap is not None:
                        nc.gpsimd.tensor_add(
                            also_swap[0:64, cs_], m1[64:128, :], m2[64:128, :])
                        nc.gpsimd.tensor_add(
                            also_swap[64:128, cs_], m1[0:64, :], m2[0:64, :])

            def v_transpose():
                V = qkvT_t[1]
                for k in range(9):
                    for g in range(KG):
                        pt = psR.tile([128, TCH], f32, tag="rot", name="vt")
                        ptb = pt.bitcast(bf16)
                        nc.tensor.transpose(
                            ptb[:, 0:D],
                            V[64 * g:64 * (g + 1), 128 * k:128 * (k + 1)],
                            identb[64 * g:64 * (g + 1), 64 * g:64 * (g + 1)])
                        nc.vector.tensor_copy(Vtok[:, k, g, :], ptb[:, 0:D])

            pb = es.enter_context(tc.tile_pool(name="phB", bufs=2))
            pbt = es.enter_context(tc.tile_pool(name="phBt", bufs=4))
            psB = psBc = psBt = None

            def attn_group(qb, g):
                """One 4-head group (kv group g) for query block qb.
                Slot order [4g, 4g+2, 4g+1, 4g+3]: each PSUM bank gets a
                same-half pair of scores matmuls."""
                sc4 = psB.tile([128, 4, 256], f32, tag="sc4", name="sc4")
                for slot in range(4):
                    half = slot // 2
                    pair = 2 * g + (slot % 2)
                    ktile = KR if (g == half) else KRsw
                    qsl = QR[pair][64 * half:64 * (half + 1), :]
                    ksl = ktile[64 * half:64 * (half + 1), :]
                    nc.tensor.matmul(
                        sc4[:, slot, :],
                        qsl[:, 128 * (qb + 1):128 * (qb + 2)],
                        ksl[:, 128 * qb:128 * qb + 256],
                        start=True, stop=True)
                pS4 = pb.tile([128, 4, 256], bf16, tag="pS4", name="pS4")
                nc.scalar.activation(out=pS4[:, 0:2, :], in_=sc4[:, 0:2, :],
                                     func=Exp)
                nc.scalar.activation(out=pS4[:, 2:4, :], in_=sc4[:, 2:4, :],
                                     func=Exp)
                pM4 = pb.tile([128, 4, 256], bf16, tag="pM4", name="pM4")
                rs4 = pb.tile([128, 4], f32, tag="rs4", name="rs4")
                mk = masks2[:, min(qb, 1), :]
                for j in range(2):
                    nc.vector.scalar_tensor_tensor(
                        out=pM4[:, j, :], in0=pS4[:, j, :], scalar=0.0,
                        in1=mk, op0=Alu.bypass, op1=Alu.mult,
                        accum_out=rs4[:, j:j + 1])
                for j in range(2, 4):
                    nc.gpsimd.tensor_mul(pM4[:, j, :], pS4[:, j, :], mk)
                nc.vector.tensor_reduce(
                    rs4[:, 2:4], pM4[:, 2:4, :], axis=mybir.AxisListType.X,
                    op=Alu.add)
                dn4 = pb.tile([128, 4], f32, tag="dn4", name="dn4")
                nc.vector.tensor_add(dn4, rs4, es_sink[:, 4 * g:4 * g + 4])
                rinv4 = pb.tile([128, 4], f32, tag="rinv4", name="rinv4")
                nc.vector.reciprocal(rinv4, dn4)
                pN4 = pb.tile([128, 4, 256], bf16, tag="pN4", name="pN4")
                nc.vector.tensor_tensor(
                    out=pN4, in0=pM4,
                    in1=rinv4.unsqueeze(2).broadcast_to([128, 4, 256]),
                    op=Alu.mult)
                pT4 = pbt.tile([128, 4, 2, 128], bf16, tag="pT4", name="pT4")
                tp4 = psBt.tile([128, 4, 2, 128], bf16, tag="tp4", name="tp4")
                for j in range(4):
                    for bk in range(2):
                        nc.tensor.transpose(
                            tp4[:, j, bk, :],
                            pN4[:, j, 128 * bk:128 * (bk + 1)], identb)
                nc.vector.tensor_copy(pT4, tp4)
                cps4 = psBc.tile([64, 4, 128], f32, tag="cps4", name="cps4")
                for j in range(4):
                    for bk in range(2):
                        nc.tensor.matmul(
                            cps4[:, j, :], Vtok[:, qb + bk, g, :],
                            pT4[:, j, bk, :],
                            start=(bk == 0), stop=(bk == 1))
                th, qq = qb // 4, qb % 4
                nc.scalar.activation(
                    out=ctxT[0:64, 2 * g:2 * g + 2, th, 128 * qq:128 * (qq + 1)],
                    in_=cps4[:, 0:2, :], func=Ident)
                nc.scalar.activation(
                    out=ctxT[64:128, 2 * g:2 * g + 2, th, 128 * qq:128 * (qq + 1)],
                    in_=cps4[:, 2:4, :], func=Ident)

            pco = es.enter_context(tc.tile_pool(name="phCo", bufs=3))
            psC = None

            def outproj():
                for eq in range(4):
                    o_sb = pco.tile([128, 4, 2, 512], bf16, tag="o", name="o")
                    for el in range(4):
                        e = 4 * eq + el
                        pst = [psC.tile([128, 512], f32, tag=f"mmC{t}",
                                        name=f"mmC{t}") for t in range(2)]
                        for h4 in range(4):
                            for t in range(2):
                                nc.tensor.matmul(
                                    pst[t], Wo[:, h4, 128 * e:128 * (e + 1)],
                                    ctxT[:, h4, t, :],
                                    start=(h4 == 0), stop=(h4 == 3))
                        nc.vector.tensor_copy(o_sb[:, el, 0, :], pst[0])
                        nc.scalar.activation(out=o_sb[:, el, 1, :],
                                             in_=pst[1], func=Ident)
                    eng = nc.sync if eq % 2 == 0 else nc.scalar
                    eng.dma_start(outT[:, 4 * eq:4 * eq + 4, :], o_sb)

            # ---------- emission schedule ----------
            qkv_block(0)                                   # K
            rope(0, cs_t["cK"], cs_t["sK"], KR, also_swap=KRsw)
            qkv_block(1)                                   # V
            v_transpose()
            qkv_block(2)                                   # Q0
            rope(2, cs_t["cQ"], cs_t["sQ"], QR[0])
            qkv_block(3)                                   # Q1
            rope(3, cs_t["cQ"], cs_t["sQ"], QR[1])
            qkv_block(4)                                   # Q2
            rope(4, cs_t["cQ"], cs_t["sQ"], QR[2])
            qkv_block(5)                                   # Q3
            rope(5, cs_t["cQ"], cs_t["sQ"], QR[3])
            esA.close()                                    # free A PSUM banks

            esB = ExitStack()
            psB = esB.enter_context(
                tc.tile_pool(name="psB", bufs=2, space="PSUM"))
            psBc = esB.enter_context(
                tc.tile_pool(name="psBc", bufs=2, space="PSUM"))
            psBt = esB.enter_context(
                tc.tile_pool(name="psBt", bufs=2, space="PSUM"))

            for qb in range(8):
                attn_group(qb, 0)
            for qb in range(8):
                attn_group(qb, 1)
            esB.close()                                    # free B PSUM banks

            psC = es.enter_context(
                tc.tile_pool(name="psC", bufs=2, space="PSUM"))
            outproj()

    nc.compile()
    return nc


# ---------------- host-side prep ----------------

def _rope_tables(position_ids, gstart):
    pos = np.zeros(TOK, dtype=np.float32)
    idx = gstart + np.arange(TOK)
    valid = (idx >= 0) & (idx < S)
    pos[valid] = position_ids[0, idx[valid]].astype(np.float32)
    freqs = (1.0 / ROPE_BASE ** (np.arange(0, D, 2, dtype=np.float32) / D)).astype(np.float32)
    wave_len = 2.0 * np.pi / freqs
    low = ORIG_CTX / BETA_FAST
    high = ORIG_CTX / BETA_SLOW
    t = np.clip((wave_len - low) / (high - low), 0.0, 1.0)
    eff = freqs * (1.0 - t) + (freqs / YARN_SCALE) * t
    conc = 0.1 * np.log(np.float32(YARN_SCALE)) + 1.0
    ang = pos[:, None] * eff[None, :] * conc
    sin = np.sin(ang).astype(np.float32).T    # [32, TOK]
    cos = np.cos(ang).astype(np.float32).T
    cosT = np.concatenate([cos, cos], axis=0)  # [64, TOK]
    sinS = np.concatenate([-sin, sin], axis=0)
    cos2 = np.concatenate([cosT, cosT], axis=0)  # [128, TOK]
    sinS2 = np.concatenate([sinS, sinS], axis=0)
    return np.ascontiguousarray(cos2), np.ascontiguousarray(sinS2)


def _build_masks01(s, gstart):
    """Multiplicative 0/1 band mask, [128, 2, 256] (qb==0 variant, qb>=1)."""
    qb = np.arange(2)[None, :, None]
    il = np.arange(128)[:, None, None]
    j = np.arange(256)[None, None, :]
    gq = 1024 * s + 128 * qb + il
    gk = gstart + 128 * qb + j
    gq_b, gk_b = np.broadcast_arrays(gq, gk)
    valid = (gk_b >= 0) & (gk_b <= gq_b) & (gk_b > gq_b - SW)
    return np.ascontiguousarray(valid.astype(np.float32))


def _perm_matrix():
    """lhsT for rotate-half: out[p] = src[p xor 32] within each 64-half."""
    P = np.zeros((128, 128), dtype=np.float32)
    for m in range(128):
        half = (m // 64) * 64
        pi = half + ((m - half) + 32) % 64
        P[pi, m] = 1.0
    return P


def _prep_core(c, x, position_ids, attn_mask, Wqkv, bqkv, Wout, sinks, xT_full):
    s, h = c // 4, c % 4
    gstart = 1024 * s - 128
    xTc = np.zeros((E, TOK), dtype=np.float32)
    lo = max(0, gstart)
    xTc[:, lo - gstart:TOK] = xT_full[:, lo:gstart + TOK]
    qrows = np.arange(512 * h, 512 * h + 512)
    krows = np.arange(H * D + 128 * h, H * D + 128 * h + 128)
    vrows = np.arange((H + G) * D + 128 * h, (H + G) * D + 128 * h + 128)
    rows = np.concatenate([krows, vrows, qrows])   # feature order K, V, Q
    WqkvTc = np.ascontiguousarray(Wqkv[rows].T)
    bq = bqkv[rows].reshape(FTOT // 128, 128).T
    WoutTc = np.ascontiguousarray(Wout[:, 512 * h:512 * h + 512].T)
    cos2, sinS2 = _rope_tables(position_ids, gstart)
    masks = _build_masks01(s, gstart)
    # slot order within each 4-head group: [4g, 4g+2, 4g+1, 4g+3]
    slot_perm = [0, 2, 1, 3, 4, 6, 5, 7]
    esink = np.ascontiguousarray(
        np.broadcast_to(np.exp(sinks[0, 8 * h:8 * h + 8, 0, 0])[slot_perm][None, :],
                        (128, QH))).astype(np.float32)
    import ml_dtypes
    bf = ml_dtypes.bfloat16
    # partition-major packing: [E, n] -> [128, E//128, n] so each DMA run
    # is contiguous per partition
    xP = xTc.reshape(NE, 128, TOK).transpose(1, 0, 2)
    wP = (WqkvTc.reshape(NE, 128, FTOT).transpose(1, 0, 2)
          .reshape(128, NE, 3, 256).transpose(0, 2, 1, 3))
    woP = WoutTc.reshape(4, 128, E).transpose(1, 0, 2)
    tabs = np.stack([cos2, sinS2, SCALE * cos2, SCALE * sinS2], axis=1)
    return {
        "xT": np.ascontiguousarray(xP.astype(bf)),
        "wqkvT": np.ascontiguousarray(wP.astype(bf)),
        "bqkvT": np.ascontiguousarray(bq.astype(np.float32)),
        "woutT": np.ascontiguousarray(woP.astype(bf)),
        "tabs": np.ascontiguousarray(tabs.astype(bf)),
        "masks": np.ascontiguousarray(masks.astype(bf)),
        "esink": esink,
        "perm": np.ascontiguousarray(_perm_matrix().astype(bf)),
    }


def _prep_all(inputs):
    x = np.asarray(inputs["x"], dtype=np.float32)
    position_ids = np.asarray(inputs["position_ids"])
    attn_mask = np.asarray(inputs["attn_mask"], dtype=np.float32)
    Wqkv = np.asarray(inputs["Wqkv"], dtype=np.float32)
    bqkv = np.asarray(inputs["bqkv"], dtype=np.float32)
    Wout = np.asarray(inputs["Wout"], dtype=np.float32)
    sinks = np.asarray(inputs["sinks"], dtype=np.float32)
    xT_full = np.ascontiguousarray(x[0].T)
    return [
        _prep_core(c, x, position_ids, attn_mask, Wqkv, bqkv, Wout, sinks, xT_full)
        for c in range(NCORES)
    ]


def kernel(x, position_ids, attn_mask, Wqkv, bqkv, Wout, bout, sinks):
    global _compiled
    from concourse.bass_utils import run_bass_kernel_spmd

    bout = np.asarray(bout, dtype=np.float32)

    if _compiled is None:
        _compiled = _build_bass()
    nc = _compiled

    in_maps = _prep_all({
        "x": x, "position_ids": position_ids, "attn_mask": attn_mask,
        "Wqkv": Wqkv, "bqkv": bqkv, "Wout": Wout, "bout": bout, "sinks": sinks,
    })
    res = run_bass_kernel_spmd(nc, in_maps, list(range(NCORES)))

    out = np.empty((S, E), dtype=np.float32)
    for s in range(2):
        acc = res.results[4 * s]["outT"].astype(np.float32)
        for h in range(1, 4):
            acc = acc + res.results[4 * s + h]["outT"].astype(np.float32)
        # [128, 16, 1024] partition-major -> [E, NQ] -> [NQ, E]
        out[1024 * s:1024 * (s + 1)] = acc.transpose(1, 0, 2).reshape(E, NQ).T
    out += bout[None, :]
    return out[None]

